# revision 1
# baseline (speedup 1.0000x reference)
"""Fused ACNet-style 5-branch conv block as a single 3x3 conv on Trainium2.

The reference computes
    out = conv3x3(x, w_square) + conv3x1(x, w_ver) + conv1x3(x, w_hor)
        + conv3x3(x, w_diag19 * eye3) + conv3x3(x, w_diag37 * antieye3)
All five branches are linear convs with identical output geometry, so they
fold into ONE effective 3x3 conv whose weight is the sum of the embedded /
masked branch weights.  The conv runs as 9 shifted matmuls (one per tap)
accumulated in PSUM, channels on the 128 SBUF partitions (C_in = C_out = 128):
    out[:, h, w] += W[kh,kw].T @ x_pad[:, h+kh, w+kw]

Input layout: spacer-packed rows — each padded row is 193 elements (192 data
+ 1 shared zero spacer).  The spacer acts as right-pad of row r AND left-pad
of row r+1, so every tap shift is a pure flat offset and each matmul's moving
operand is ONE contiguous 386-element run (2 output rows per PSUM bank).
Matmul operands are tagged float32r: full fp32 storage, reduced-precision
multiply at 1 row/cycle (4x faster than float32 mode, ~1.4e-4 rel err).

Sharding: data-parallel over batch — 16 images / 8 cores = 2 images per
core, weights replicated, no collectives.
"""

import sys

for _p in ("/opt/trn_rl_repo",):
    if _p not in sys.path:
        sys.path.insert(0, _p)

import numpy as np

import concourse.mybir as mybir
import concourse.tile as tile
from concourse import bacc
from concourse.bass_utils import run_bass_kernel_spmd

B, C, H, W = 16, 128, 192, 192
NCORES = 8
IPC = B // NCORES  # images per core
NTAP = 9
SW = W + 1  # spacer-packed row width (193)
XLEN = 1 + (H + 2) * SW + 4  # leading zero + 194 packed rows + tap margin
RB = 32  # output rows per block
MM_DT = mybir.dt.float32r


def _build(ipc, rb, mm_dt, repeat=1, xbufs=3, obufs=2, ahead=1):
    """Emit the per-core Bass program.

    The x-DMA for block k+ahead is issued before block k's compute/out-DMA
    in program order, so input prefetch never queues behind output drains.
    repeat>1 wraps the body in a For_i loop (timing harness only; the body
    is idempotent so outputs are unchanged).
    """
    nc = bacc.Bacc("TRN2", target_bir_lowering=False, debug=False)
    x_in = nc.dram_tensor(
        "x", [ipc, C, XLEN], mybir.dt.float32, kind="ExternalInput"
    ).ap()
    w_in = nc.dram_tensor(
        "w", [C, NTAP * C], mybir.dt.float32, kind="ExternalInput"
    ).ap()
    y_out = nc.dram_tensor(
        "y", [ipc, C, H, W], mybir.dt.float32, kind="ExternalOutput"
    ).ap()

    xtl = (rb + 2) * SW + 4  # x tile flat length per partition
    blocks = [(img, r0) for img in range(ipc) for r0 in range(0, H, rb)]

    with tile.TileContext(nc) as tc:
        with (
            tc.tile_pool(name="wp", bufs=1) as wpool,
            tc.tile_pool(name="xp", bufs=xbufs) as xpool,
            tc.tile_pool(name="op", bufs=obufs) as opool,
            tc.tile_pool(name="ps", bufs=8, space="PSUM") as pspool,
        ):
            # fp32r matmul operands must be produced as fp32r; a dtype-tagged
            # DMA qualifies (pure bitcast of the fp32 data).
            wt = wpool.tile([C, NTAP * C], mm_dt)
            nc.sync.dma_start(wt[:], w_in[:].bitcast(mm_dt))

            def load(img, r0):
                xt = xpool.tile([C, xtl], mm_dt, tag="xt", name=f"xt{img}_{r0}")
                base = r0 * SW
                nc.sync.dma_start(
                    xt[:], x_in[img, :, base : base + xtl].bitcast(mm_dt)
                )
                return xt

            def body():
                xts = [load(*blocks[k]) for k in range(min(ahead, len(blocks)))]
                for k, (img, r0) in enumerate(blocks):
                    if k + ahead < len(blocks):
                        xts.append(load(*blocks[k + ahead]))
                    xt = xts.pop(0)
                    ot = opool.tile([C, rb, W], mybir.dt.float32, tag="ot",
                                    name=f"ot{img}_{r0}")
                    for p in range(rb // 2):
                        ps = pspool.tile([C, 2 * SW], mybir.dt.float32,
                                         tag="ps", name=f"ps{p}")
                        for t in range(NTAP):
                            kh, kw = divmod(t, 3)
                            off = (2 * p + kh) * SW + kw
                            nc.tensor.matmul(
                                ps[:],
                                wt[:, t * C : (t + 1) * C],
                                xt[:, off : off + 2 * SW],
                                start=(t == 0),
                                stop=(t == NTAP - 1),
                            )
                        # strip the spacer columns while draining PSUM
                        eng = nc.scalar.copy if p % 2 == 0 else (
                            nc.vector.tensor_copy
                        )
                        eng(ot[:, 2 * p, :], ps[:, 0:W])
                        eng(ot[:, 2 * p + 1, :], ps[:, SW : SW + W])
                    nc.sync.dma_start(y_out[img, :, r0 : r0 + rb, :], ot[:])

            if repeat == 1:
                body()
            else:
                with tc.For_i(0, repeat, 1):
                    body()
    nc.compile()
    return nc


def _fold_weights(w_square, w_ver, w_hor, w_diag19, w_diag37):
    """Fold the 5 branches into one 3x3 weight, laid out [C_in, tap*C_out]."""
    eye = np.eye(3, dtype=np.float32)
    anti = eye[::-1, :]
    w_eff = (
        np.asarray(w_square, np.float32)
        + np.asarray(w_diag19, np.float32) * eye
        + np.asarray(w_diag37, np.float32) * anti
    )
    w_eff[:, :, :, 1] += np.asarray(w_ver, np.float32)[:, :, :, 0]
    w_eff[:, :, 1, :] += np.asarray(w_hor, np.float32)[:, :, 0, :]
    # [O, I, KH, KW] -> [I, KH, KW, O] -> [I, (KH*KW)*O]  (lhsT per tap)
    return np.ascontiguousarray(w_eff.transpose(1, 2, 3, 0).reshape(C, NTAP * C))


def _pack_x(x):
    """[B,C,H,W] -> spacer-packed flat [B,C,XLEN]."""
    xs = np.zeros((B, C, XLEN), np.float32)
    rows = xs[:, :, 1 : 1 + (H + 2) * SW].reshape(B, C, H + 2, SW)
    rows[:, :, 1 : H + 1, 0:W] = x
    return xs


_nc_cache = {}


def kernel(x, w_square, w_ver, w_hor, w_diag19, w_diag37):
    x = np.asarray(x, np.float32)
    w_host = _fold_weights(w_square, w_ver, w_hor, w_diag19, w_diag37)
    xs = _pack_x(x)

    if "nc" not in _nc_cache:
        _nc_cache["nc"] = _build(IPC, RB, MM_DT)
    nc = _nc_cache["nc"]

    in_maps = [
        {"x": np.ascontiguousarray(xs[c * IPC : (c + 1) * IPC]), "w": w_host}
        for c in range(NCORES)
    ]
    res = run_bass_kernel_spmd(nc, in_maps, list(range(NCORES)))
    return np.concatenate([res.results[c]["y"] for c in range(NCORES)], axis=0)



# revision 2
# speedup vs baseline: 1.2304x; 1.2304x over previous
"""Fused ACNet-style 5-branch conv block as a single 3x3 conv on Trainium2.

The reference computes
    out = conv3x3(x, w_square) + conv3x1(x, w_ver) + conv1x3(x, w_hor)
        + conv3x3(x, w_diag19 * eye3) + conv3x3(x, w_diag37 * antieye3)
All five branches are linear convs with identical output geometry, so they
fold into ONE effective 3x3 conv whose weight is the sum of the embedded /
masked branch weights.  The conv runs as 9 shifted matmuls (one per tap)
accumulated in PSUM, channels on the 128 SBUF partitions (C_in = C_out = 128):
    out[:, h, w] += W[kh,kw].T @ x_pad[:, h+kh, w+kw]

Input layout: spacer-packed rows — each padded row is 193 elements (192 data
+ 1 shared zero spacer), so every tap shift is a pure flat offset and each
matmul's moving operand is ONE contiguous 386-element run (2 output rows per
PSUM bank).  Matmul operands are float32r (full fp32 storage, reduced-
precision multiply at 1 row/cycle).

Loop order: taps OUTER over a superblock of 8 PSUM banks (16 output rows),
so each stationary weight load serves 8 consecutive matmuls — measured
~1.23x over the taps-inner order (the fp32r weight reload is not hidden by
the PE reorder window when the stationary changes every matmul).

Sharding: data-parallel over batch — 16 images / 8 cores = 2 images per
core, weights replicated, no collectives.
"""

import sys

for _p in ("/opt/trn_rl_repo",):
    if _p not in sys.path:
        sys.path.insert(0, _p)

import numpy as np

import concourse.mybir as mybir
import concourse.tile as tile
from concourse import bacc
from concourse.bass_utils import run_bass_kernel_spmd

B, C, H, W = 16, 128, 192, 192
NCORES = 8
IPC = B // NCORES  # images per core
NTAP = 9
SW = W + 1  # spacer-packed row width (193)
XLEN = 1 + (H + 2) * SW + 4  # leading zero + 194 packed rows + tap margin
SB = 16  # output rows per superblock (8 PSUM banks x 2 rows)
NBANK = SB // 2
TAP_OFF = [(t // 3) * SW + (t % 3) for t in range(NTAP)]


def _build(repeat=1, ipc=IPC):
    """Emit the per-core Bass program (fp32r, taps-outer)."""
    nc = bacc.Bacc("TRN2", target_bir_lowering=False, debug=False)
    mm_dt = mybir.dt.float32r
    x_in = nc.dram_tensor(
        "x", [ipc, C, XLEN], mybir.dt.float32, kind="ExternalInput"
    ).ap()
    w_in = nc.dram_tensor(
        "w", [C, NTAP * C], mybir.dt.float32, kind="ExternalInput"
    ).ap()
    y_out = nc.dram_tensor(
        "y", [ipc, C, H, W], mybir.dt.float32, kind="ExternalOutput"
    ).ap()

    xtl = (SB + 2) * SW + 4
    blocks = [(img, r0) for img in range(ipc) for r0 in range(0, H, SB)]

    with tile.TileContext(nc) as tc:
        with (
            tc.tile_pool(name="wp", bufs=1) as wpool,
            tc.tile_pool(name="xp", bufs=3) as xpool,
            tc.tile_pool(name="op", bufs=2) as opool,
            tc.tile_pool(name="ps", bufs=8, space="PSUM") as pspool,
        ):
            wt = wpool.tile([C, NTAP, C], mm_dt)
            nc.sync.dma_start(wt[:], w_in[:].bitcast(mm_dt))

            def load(img, r0):
                xt = xpool.tile([C, xtl], mm_dt, tag="xt", name=f"xt{img}_{r0}")
                base = r0 * SW
                nc.sync.dma_start(
                    xt[:], x_in[img, :, base : base + xtl].bitcast(mm_dt)
                )
                return xt

            def body():
                xts = [load(*blocks[k]) for k in range(1)]
                for k, (img, r0) in enumerate(blocks):
                    if k + 1 < len(blocks):
                        xts.append(load(*blocks[k + 1]))
                    xt = xts.pop(0)
                    ot = opool.tile([C, SB, W], mybir.dt.float32, tag="ot",
                                    name=f"ot{img}_{r0}")
                    pss = [
                        pspool.tile([C, 2, SW], mybir.dt.float32, tag="ps",
                                    name=f"ps{p}")
                        for p in range(NBANK)
                    ]
                    for t in range(NTAP):
                        lhsT = wt[:, t : t + 1, :].squeeze(1)
                        off = TAP_OFF[t]
                        for p in range(NBANK):
                            o = off + 2 * p * SW
                            nc.tensor.matmul(
                                pss[p][:],
                                lhsT,
                                xt[:, o : o + 2 * SW],
                                start=(t == 0),
                                stop=(t == NTAP - 1),
                            )
                    for p in range(NBANK):
                        dst = ot[:, 2 * p : 2 * p + 2, :]
                        src = pss[p][:, :, 0:W]
                        if p % 2 == 0:
                            nc.scalar.copy(dst, src)
                        else:
                            nc.vector.tensor_copy(dst, src)
                    nc.sync.dma_start(y_out[img, :, r0 : r0 + SB, :], ot[:])

            if repeat == 1:
                body()
            else:
                with tc.For_i(0, repeat, 1):
                    body()
    nc.compile()
    return nc


def _fold_weights(w_square, w_ver, w_hor, w_diag19, w_diag37):
    """Fold the 5 branches into one 3x3 weight, laid out [C_in, tap*C_out]."""
    eye = np.eye(3, dtype=np.float32)
    anti = eye[::-1, :]
    w_eff = (
        np.asarray(w_square, np.float32)
        + np.asarray(w_diag19, np.float32) * eye
        + np.asarray(w_diag37, np.float32) * anti
    )
    w_eff[:, :, :, 1] += np.asarray(w_ver, np.float32)[:, :, :, 0]
    w_eff[:, :, 1, :] += np.asarray(w_hor, np.float32)[:, :, 0, :]
    # [O, I, KH, KW] -> [I, KH, KW, O] -> [I, (KH*KW)*O]  (lhsT per tap)
    return np.ascontiguousarray(w_eff.transpose(1, 2, 3, 0).reshape(C, NTAP * C))


def _pack_x(x):
    """[B,C,H,W] -> spacer-packed flat [B,C,XLEN]."""
    xs = np.zeros((B, C, XLEN), np.float32)
    rows = xs[:, :, 1 : 1 + (H + 2) * SW].reshape(B, C, H + 2, SW)
    rows[:, :, 1 : H + 1, 0:W] = x
    return xs


def _prepare(x, w_square, w_ver, w_hor, w_diag19, w_diag37):
    w_host = _fold_weights(w_square, w_ver, w_hor, w_diag19, w_diag37)
    xs = _pack_x(np.asarray(x, np.float32))
    return [
        {"x": np.ascontiguousarray(xs[c * IPC : (c + 1) * IPC]), "w": w_host}
        for c in range(NCORES)
    ]


_nc_cache = {}


def kernel(x, w_square, w_ver, w_hor, w_diag19, w_diag37):
    in_maps = _prepare(x, w_square, w_ver, w_hor, w_diag19, w_diag37)
    if "nc" not in _nc_cache:
        _nc_cache["nc"] = _build()
    nc = _nc_cache["nc"]
    res = run_bass_kernel_spmd(nc, in_maps, list(range(NCORES)))
    return np.concatenate([res.results[c]["y"] for c in range(NCORES)], axis=0)


# revision 3
# speedup vs baseline: 1.3662x; 1.1104x over previous
"""Fused ACNet-style 5-branch conv block on Trainium2 via 1-D Winograd.

The reference computes
    out = conv3x3(x, w_square) + conv3x1(x, w_ver) + conv1x3(x, w_hor)
        + conv3x3(x, w_diag19 * eye3) + conv3x3(x, w_diag37 * antieye3)
All five branches are linear convs with identical output geometry, so they
fold into ONE effective 3x3 conv (weights summed host-side).

The 3x3 conv runs as 3 row-convs of 1x3, each via Winograd F(2,3) along the
width: for an output pair (2j, 2j+1) and kernel row g = (g0,g1,g2),
    t0 = d0-d2, t1 = d1+d2, t2 = d2-d1, t3 = d1-d3   (d = x[2j-1 .. 2j+2])
    m_xi = U_xi @ t_xi with U = (g0, (g0+g1+g2)/2, (g0-g1+g2)/2, g2)
    even = m0+m1+m2,  odd = m1-m2-m3
This needs 6 channel-contractions per output pixel instead of the direct
method's 9, cutting PE cycles 1.5x.  The kh-sum accumulates in PSUM (the
t-stream of row r+kh is the same flat stream shifted by 96 windows), so
each 512-window PSUM tile takes 12 matmuls (4 xi x 3 kh) of N=512 bf16.

Input layout: per padded row, even/odd column planes [E(97) | O(97)], so
every transform input is a stride-1 run (DVE 2x-eligible in bf16).  The
transforms run on the DVE, emitted one superblock ahead of the matmuls
that consume them; the xi-combination doubles as the PSUM drain, with m1/m2
staged through the otherwise-idle ACT engine (tensor_tensor reads at most
one PSUM operand).  Output is written f16 and widened to f32 on the host.

Measured: 306 us vs 399 us for the best direct fp32r kernel (9 shifted
matmuls, taps-outer) and 454 us for the staged baseline; bf16 rel_absmax
3.3e-3 (tolerance 2e-2).

Sharding: data-parallel over batch - 16 images / 8 cores = 2 images per
core, weights replicated, no collectives.
"""

import sys

for _p in ("/opt/trn_rl_repo",):
    if _p not in sys.path:
        sys.path.insert(0, _p)

import ml_dtypes
import numpy as np

import concourse.mybir as mybir
import concourse.tile as tile
from concourse import bacc
from concourse.bass_utils import run_bass_kernel_spmd

B, C, H, W = 16, 128, 192, 192
NCORES = 8
IPC = B // NCORES       # images per core
WSB = 32                # output rows per superblock
TR = WSB + 2            # transformed rows held per superblock
NW = 96                 # windows (output pairs) per row
XROW = 194              # packed row: 97 even cols | 97 odd cols
WBLK = 512              # windows per PSUM block

WINO_IN_T = [
    # (in0_col, in1_col, op): t_xi[j] = x[.., in0+j] op x[.., in1+j]
    (97, 98, "subtract"),   # t0 = O[j]   - O[j+1]
    (0, 98, "add"),         # t1 = E[j]   + O[j+1]
    (98, 0, "subtract"),    # t2 = O[j+1] - E[j]
    (0, 1, "subtract"),     # t3 = E[j]   - E[j+1]
]


def _build(repeat=1, ipc=IPC):
    mm_dt = mybir.dt.bfloat16
    nc = bacc.Bacc("TRN2", target_bir_lowering=False, debug=False)
    x_in = nc.dram_tensor("x", [ipc, C, H + 2, XROW], mm_dt,
                          kind="ExternalInput").ap()
    w_in = nc.dram_tensor("w", [C, 12 * C], mm_dt, kind="ExternalInput").ap()
    y_out = nc.dram_tensor("y", [ipc, C, H, W], mybir.dt.float16,
                           kind="ExternalOutput").ap()

    blocks = [(img, r0) for img in range(ipc) for r0 in range(0, H, WSB)]
    sub = mybir.AluOpType.subtract
    add = mybir.AluOpType.add

    with tile.TileContext(nc) as tc:
        with (
            tc.tile_pool(name="wp", bufs=1) as wpool,
            tc.tile_pool(name="xp", bufs=3) as xpool,
            tc.tile_pool(name="tp", bufs=2) as tpool,
            tc.tile_pool(name="op", bufs=2) as opool,
            tc.tile_pool(name="tm", bufs=4) as tmpool,
            tc.tile_pool(name="ps", bufs=8, space="PSUM") as pspool,
        ):
            wt = wpool.tile([C, 12, C], mm_dt)
            nc.sync.dma_start(wt[:], w_in[:])

            def load(img, r0):
                xt = xpool.tile([C, TR, XROW], mm_dt, tag="xt",
                                name=f"xt{img}_{r0}")
                nc.sync.dma_start(xt[:], x_in[img, :, r0 : r0 + TR, :])
                return xt

            def trans(xt, img, r0):
                tt = tpool.tile([C, 4, TR * NW], mm_dt, tag="tt",
                                name=f"tt{img}_{r0}")
                for xi, (c0, c1, opname) in enumerate(WINO_IN_T):
                    nc.vector.tensor_tensor(
                        tt[:, xi, :].rearrange("c (r j) -> c r j", j=NW),
                        xt[:, :, c0 : c0 + NW],
                        xt[:, :, c1 : c1 + NW],
                        getattr(mybir.AluOpType, opname),
                    )
                return tt

            def body():
                xts = [load(*blocks[0])]
                tts = [trans(xts[0], *blocks[0])]
                for k, (img, r0) in enumerate(blocks):
                    if k + 1 < len(blocks):
                        xts.append(load(*blocks[k + 1]))
                        # next superblock's transform goes ahead of this
                        # one's matmuls: keeps the DVE FIFO ahead of the PE
                        tts.append(trans(xts[1], *blocks[k + 1]))
                    xt = xts.pop(0)
                    tt = tts.pop(0)
                    ot = opool.tile([C, WSB, W], mybir.dt.float16, tag="ot",
                                    name=f"ot{img}_{r0}")
                    ot_pairs = ot.rearrange("c r w -> c (r w)").rearrange(
                        "c (n t) -> c n t", t=2
                    )
                    for blk in range(WSB * NW // WBLK):
                        wl0 = blk * WBLK
                        ms = [
                            pspool.tile([C, WBLK], mybir.dt.float32, tag="ps",
                                        name=f"m{blk}_{xi}")
                            for xi in range(4)
                        ]
                        for xi in range(4):
                            for kh in range(3):
                                lhsT = wt[:, kh * 4 + xi : kh * 4 + xi + 1,
                                          :].squeeze(1)
                                rhs = tt[:, xi,
                                         wl0 + kh * NW : wl0 + kh * NW + WBLK]
                                nc.tensor.matmul(
                                    ms[xi][:], lhsT, rhs,
                                    start=(kh == 0), stop=(kh == 2),
                                )
                        # output transform == PSUM drain.  tensor_tensor
                        # reads at most one PSUM operand: stage m1/m2 via
                        # the otherwise-idle ACT engine.
                        s1 = tmpool.tile([C, WBLK], mybir.dt.float32, tag="s1")
                        s2 = tmpool.tile([C, WBLK], mybir.dt.float32, tag="s2")
                        te = tmpool.tile([C, WBLK], mybir.dt.float32, tag="te")
                        to = tmpool.tile([C, WBLK], mybir.dt.float32, tag="to")
                        ev = ot_pairs[:, wl0 : wl0 + WBLK, 0:1].squeeze(2)
                        od = ot_pairs[:, wl0 : wl0 + WBLK, 1:2].squeeze(2)
                        nc.scalar.copy(s1[:], ms[1][:])
                        nc.scalar.copy(s2[:], ms[2][:])
                        nc.vector.tensor_tensor(te[:], s1[:], ms[0][:], add)
                        nc.vector.tensor_tensor(ev, te[:], s2[:], add)
                        nc.vector.tensor_tensor(to[:], s1[:], s2[:], sub)
                        nc.vector.tensor_tensor(od, to[:], ms[3][:], sub)
                    nc.sync.dma_start(y_out[img, :, r0 : r0 + WSB, :], ot[:])

            if repeat == 1:
                body()
            else:
                with tc.For_i(0, repeat, 1):
                    body()
    nc.compile()
    return nc


def _fold_weights(w_square, w_ver, w_hor, w_diag19, w_diag37):
    eye = np.eye(3, dtype=np.float32)
    anti = eye[::-1, :]
    w_eff = (
        np.asarray(w_square, np.float32)
        + np.asarray(w_diag19, np.float32) * eye
        + np.asarray(w_diag37, np.float32) * anti
    )
    w_eff[:, :, :, 1] += np.asarray(w_ver, np.float32)[:, :, :, 0]
    w_eff[:, :, 1, :] += np.asarray(w_hor, np.float32)[:, :, 0, :]
    return w_eff  # [O, I, KH, KW]


def _pack_w(w_eff):
    """U blocks per (kh, xi), transposed to [C_in, C_out], -> [C, 12*C]."""
    blocks = []
    for kh in range(3):
        g0 = w_eff[:, :, kh, 0]
        g1 = w_eff[:, :, kh, 1]
        g2 = w_eff[:, :, kh, 2]
        for u in (g0, (g0 + g1 + g2) * 0.5, (g0 - g1 + g2) * 0.5, g2):
            blocks.append(np.ascontiguousarray(u.T.astype(np.float32)))
    return np.ascontiguousarray(
        np.concatenate(blocks, axis=1)
    ).astype(ml_dtypes.bfloat16)


def _pack_x(x):
    """[B,C,H,W] -> [B, C, H+2, 194] bf16, even/odd column planes per row."""
    xs = np.zeros((B, C, H + 2, XROW), ml_dtypes.bfloat16)
    xd = np.asarray(x, np.float32).astype(ml_dtypes.bfloat16)
    # dram row R holds padded row R-1; rows 0 and H+1 stay zero
    xs[:, :, 1 : H + 1, 0:96] = xd[:, :, :, 0::2]     # E[0..95]: cols 0,2,..,190
    xs[:, :, 1 : H + 1, 98:194] = xd[:, :, :, 1::2]   # O[1..96]: cols 1,3,..,191
    # E[96] (right pad) and O[0] (left pad) stay zero
    return xs


def _prepare(x, w_square, w_ver, w_hor, w_diag19, w_diag37):
    w_host = _pack_w(_fold_weights(w_square, w_ver, w_hor, w_diag19, w_diag37))
    xs = _pack_x(x)
    return [
        {"x": np.ascontiguousarray(xs[c * IPC : (c + 1) * IPC]), "w": w_host}
        for c in range(NCORES)
    ]


_nc_cache = {}


def kernel(x, w_square, w_ver, w_hor, w_diag19, w_diag37):
    in_maps = _prepare(x, w_square, w_ver, w_hor, w_diag19, w_diag37)
    if "nc" not in _nc_cache:
        _nc_cache["nc"] = _build()
    nc = _nc_cache["nc"]
    res = run_bass_kernel_spmd(nc, in_maps, list(range(NCORES)))
    return np.concatenate(
        [res.results[c]["y"] for c in range(NCORES)], axis=0
    ).astype(np.float32)


# revision 4
# speedup vs baseline: 1.4327x; 1.0486x over previous
"""Fused ACNet-style 5-branch conv block on Trainium2 via 1-D Winograd.

The reference computes
    out = conv3x3(x, w_square) + conv3x1(x, w_ver) + conv1x3(x, w_hor)
        + conv3x3(x, w_diag19 * eye3) + conv3x3(x, w_diag37 * antieye3)
All five branches are linear convs with identical output geometry, so they
fold into ONE effective 3x3 conv (weights summed host-side).

The 3x3 conv runs as 3 row-convs of 1x3, each via Winograd F(2,3) along the
width: for an output pair (2j, 2j+1) and kernel row g = (g0,g1,g2),
    t0 = d0-d2, t1 = d1+d2, t2 = d2-d1, t3 = d1-d3   (d = x[2j-1 .. 2j+2])
    m_xi = U_xi @ t_xi with U = (g0, (g0+g1+g2)/2, (g0-g1+g2)/2, g2)
    even = m0+m1+m2,  odd = m1-m2-m3
This needs 6 channel-contractions per output pixel instead of the direct
method's 9, cutting PE cycles 1.5x.  The kh-sum accumulates in PSUM (the
t-stream of row r+kh is the same flat stream shifted by 96 windows), so
each 512-window PSUM tile takes 12 matmuls (4 xi x 3 kh) of N=512 bf16.

Input layout: per padded row, even/odd column planes [E(97) | O(97)], so
every transform input is a stride-1 run (DVE 2x-eligible in bf16).  The
transforms run on the DVE, emitted one superblock ahead of the matmuls
that consume them; the xi-combination doubles as the PSUM drain, with m1/m2
staged through the otherwise-idle ACT engine (tensor_tensor reads at most
one PSUM operand).  Output is written f16 and widened to f32 on the host.

Measured: 306 us vs 399 us for the best direct fp32r kernel (9 shifted
matmuls, taps-outer) and 454 us for the staged baseline; bf16 rel_absmax
3.3e-3 (tolerance 2e-2).

Sharding: data-parallel over batch - 16 images / 8 cores = 2 images per
core, weights replicated, no collectives.
"""

import sys

for _p in ("/opt/trn_rl_repo",):
    if _p not in sys.path:
        sys.path.insert(0, _p)

import ml_dtypes
import numpy as np

import concourse.mybir as mybir
import concourse.tile as tile
from concourse import bacc
from concourse.bass_utils import run_bass_kernel_spmd

B, C, H, W = 16, 128, 192, 192
NCORES = 8
IPC = B // NCORES       # images per core
WSB = 32                # output rows per superblock
TR = WSB + 2            # transformed rows held per superblock
NW = 96                 # windows (output pairs) per row
XROW = 194              # packed row: 97 even cols | 97 odd cols
WBLK = 512              # windows per PSUM block

WINO_IN_T = [
    # (in0_col, in1_col, op): t_xi[j] = x[.., in0+j] op x[.., in1+j]
    (97, 98, "subtract"),   # t0 = O[j]   - O[j+1]
    (0, 98, "add"),         # t1 = E[j]   + O[j+1]
    (98, 0, "subtract"),    # t2 = O[j+1] - E[j]
    (0, 1, "subtract"),     # t3 = E[j]   - E[j+1]
]


def _build(repeat=1, ipc=IPC):
    mm_dt = mybir.dt.bfloat16
    nc = bacc.Bacc("TRN2", target_bir_lowering=False, debug=False)
    x_in = nc.dram_tensor("x", [ipc, C, H + 2, XROW], mm_dt,
                          kind="ExternalInput").ap()
    w_in = nc.dram_tensor("w", [C, 12 * C], mm_dt, kind="ExternalInput").ap()
    y_out = nc.dram_tensor("y", [ipc, C, H, W], mybir.dt.float16,
                           kind="ExternalOutput").ap()

    blocks = [(img, r0) for img in range(ipc) for r0 in range(0, H, WSB)]
    sub = mybir.AluOpType.subtract
    add = mybir.AluOpType.add

    with tile.TileContext(nc) as tc:
        with (
            tc.tile_pool(name="wp", bufs=1) as wpool,
            tc.tile_pool(name="xp", bufs=3) as xpool,
            tc.tile_pool(name="tp", bufs=2) as tpool,
            tc.tile_pool(name="op", bufs=2) as opool,
            tc.tile_pool(name="tm", bufs=4) as tmpool,
            tc.tile_pool(name="ps", bufs=8, space="PSUM") as pspool,
        ):
            wt = wpool.tile([C, 12, C], mm_dt)
            nc.sync.dma_start(wt[:], w_in[:])

            def load(img, r0):
                xt = xpool.tile([C, TR, XROW], mm_dt, tag="xt",
                                name=f"xt{img}_{r0}")
                nc.sync.dma_start(xt[:], x_in[img, :, r0 : r0 + TR, :])
                return xt

            def trans(xt, img, r0):
                tt = tpool.tile([C, 4, TR * NW], mm_dt, tag="tt",
                                name=f"tt{img}_{r0}")
                for xi, (c0, c1, opname) in enumerate(WINO_IN_T):
                    nc.vector.tensor_tensor(
                        tt[:, xi, :].rearrange("c (r j) -> c r j", j=NW),
                        xt[:, :, c0 : c0 + NW],
                        xt[:, :, c1 : c1 + NW],
                        getattr(mybir.AluOpType, opname),
                    )
                return tt

            def body():
                xts = [load(*blocks[0])]
                tts = [trans(xts[0], *blocks[0])]
                for k, (img, r0) in enumerate(blocks):
                    if k + 1 < len(blocks):
                        xts.append(load(*blocks[k + 1]))
                        # next superblock's transform goes ahead of this
                        # one's matmuls: keeps the DVE FIFO ahead of the PE
                        tts.append(trans(xts[1], *blocks[k + 1]))
                    xt = xts.pop(0)
                    tt = tts.pop(0)
                    ot = opool.tile([C, WSB, W], mybir.dt.float16, tag="ot",
                                    name=f"ot{img}_{r0}")
                    ot_pairs = ot.rearrange("c r w -> c (r w)").rearrange(
                        "c (n t) -> c n t", t=2
                    )
                    # block PAIRS: each stationary load serves 2 matmuls,
                    # and the 8 PSUM banks hold both blocks' 4 m-tiles.
                    for q in range(WSB * NW // WBLK // 2):
                        blks = (2 * q, 2 * q + 1)
                        ms = [
                            pspool.tile([C, WBLK], mybir.dt.float32, tag="ps",
                                        name=f"m{blk}_{xi}")
                            for blk in blks for xi in range(4)
                        ]
                        for xi in range(4):
                            for kh in range(3):
                                lhsT = wt[:, kh * 4 + xi : kh * 4 + xi + 1,
                                          :].squeeze(1)
                                for b, blk in enumerate(blks):
                                    wl0 = blk * WBLK
                                    rhs = tt[:, xi,
                                             wl0 + kh * NW : wl0 + kh * NW + WBLK]
                                    nc.tensor.matmul(
                                        ms[b * 4 + xi][:], lhsT, rhs,
                                        start=(kh == 0), stop=(kh == 2),
                                    )
                        # output transform == PSUM drain.  tensor_tensor
                        # reads at most one PSUM operand: stage m1/m2 via
                        # the otherwise-idle ACT engine; interleave blocks
                        # so banks free in the order the next group's
                        # matmuls consume them.
                        tl = {}
                        for nm in ("s1", "s2", "te", "to"):
                            for b in range(2):
                                tl[nm, b] = tmpool.tile(
                                    [C, WBLK], mybir.dt.float32,
                                    tag=f"{nm}{b}", name=f"{nm}{b}_{q}"
                                )
                        for b in range(2):
                            nc.scalar.copy(tl["s1", b][:], ms[b * 4 + 1][:])
                        for b in range(2):
                            nc.scalar.copy(tl["s2", b][:], ms[b * 4 + 2][:])
                        for b in range(2):
                            nc.vector.tensor_tensor(
                                tl["te", b][:], tl["s1", b][:],
                                ms[b * 4 + 0][:], add)
                        for b, blk in enumerate(blks):
                            ev = ot_pairs[:, blk * WBLK : blk * WBLK + WBLK,
                                          0:1].squeeze(2)
                            nc.vector.tensor_tensor(
                                ev, tl["te", b][:], tl["s2", b][:], add)
                        for b in range(2):
                            nc.vector.tensor_tensor(
                                tl["to", b][:], tl["s1", b][:],
                                tl["s2", b][:], sub)
                        for b, blk in enumerate(blks):
                            od = ot_pairs[:, blk * WBLK : blk * WBLK + WBLK,
                                          1:2].squeeze(2)
                            nc.vector.tensor_tensor(
                                od, tl["to", b][:], ms[b * 4 + 3][:], sub)
                    nc.sync.dma_start(y_out[img, :, r0 : r0 + WSB, :], ot[:])

            if repeat == 1:
                body()
            else:
                with tc.For_i(0, repeat, 1):
                    body()
    nc.compile()
    return nc


def _fold_weights(w_square, w_ver, w_hor, w_diag19, w_diag37):
    eye = np.eye(3, dtype=np.float32)
    anti = eye[::-1, :]
    w_eff = (
        np.asarray(w_square, np.float32)
        + np.asarray(w_diag19, np.float32) * eye
        + np.asarray(w_diag37, np.float32) * anti
    )
    w_eff[:, :, :, 1] += np.asarray(w_ver, np.float32)[:, :, :, 0]
    w_eff[:, :, 1, :] += np.asarray(w_hor, np.float32)[:, :, 0, :]
    return w_eff  # [O, I, KH, KW]


def _pack_w(w_eff):
    """U blocks per (kh, xi), transposed to [C_in, C_out], -> [C, 12*C]."""
    blocks = []
    for kh in range(3):
        g0 = w_eff[:, :, kh, 0]
        g1 = w_eff[:, :, kh, 1]
        g2 = w_eff[:, :, kh, 2]
        for u in (g0, (g0 + g1 + g2) * 0.5, (g0 - g1 + g2) * 0.5, g2):
            blocks.append(np.ascontiguousarray(u.T.astype(np.float32)))
    return np.ascontiguousarray(
        np.concatenate(blocks, axis=1)
    ).astype(ml_dtypes.bfloat16)


def _pack_x(x):
    """[B,C,H,W] -> [B, C, H+2, 194] bf16, even/odd column planes per row."""
    xs = np.zeros((B, C, H + 2, XROW), ml_dtypes.bfloat16)
    xd = np.asarray(x, np.float32).astype(ml_dtypes.bfloat16)
    # dram row R holds padded row R-1; rows 0 and H+1 stay zero
    xs[:, :, 1 : H + 1, 0:96] = xd[:, :, :, 0::2]     # E[0..95]: cols 0,2,..,190
    xs[:, :, 1 : H + 1, 98:194] = xd[:, :, :, 1::2]   # O[1..96]: cols 1,3,..,191
    # E[96] (right pad) and O[0] (left pad) stay zero
    return xs


def _prepare(x, w_square, w_ver, w_hor, w_diag19, w_diag37):
    w_host = _pack_w(_fold_weights(w_square, w_ver, w_hor, w_diag19, w_diag37))
    xs = _pack_x(x)
    return [
        {"x": np.ascontiguousarray(xs[c * IPC : (c + 1) * IPC]), "w": w_host}
        for c in range(NCORES)
    ]


_nc_cache = {}


def kernel(x, w_square, w_ver, w_hor, w_diag19, w_diag37):
    in_maps = _prepare(x, w_square, w_ver, w_hor, w_diag19, w_diag37)
    if "nc" not in _nc_cache:
        _nc_cache["nc"] = _build()
    nc = _nc_cache["nc"]
    res = run_bass_kernel_spmd(nc, in_maps, list(range(NCORES)))
    return np.concatenate(
        [res.results[c]["y"] for c in range(NCORES)], axis=0
    ).astype(np.float32)
